# revision 7
# baseline (speedup 1.0000x reference)
"""Multi-positive InfoNCE contrastive loss on 8 Trainium2 NeuronCores.

Math (matches the reference):
    x      = e / ||e||                               (L2 row normalize)
    logits = (x @ x.T) / T,  diag excluded
    loss   = sum_i [ n_i * logZ_i - sum_{j in P_i} logits_ij ] / B

Device strategy (per core c, SPMD over 8 cores, no collectives):
  * Host ships E^T quantized to fp8e4 (full [D,B] + own-column shard
    [D,B/8]) and the mask row-shard as bf16 {0,1}.  n_i is folded into the
    host-side scalar reduction.
  * Raw gram G = eq^T eq via fp8 DoubleRow matmuls (2 k-planes per
    instruction).  Normalization is applied POST-matmul:
      logits_ij = G_ij * (T^-1 u_i u_j),  u = 1/||eq||
    - r2_j = ||eq_j||^2 for all j from per-128-column-block diagonal gram
      matmuls (DoubleRow) + identity-STT diag extraction -> compact [128,nb].
    - compact r2 -> [1, cols] row via SBUF-to-SBUF DMA rearrange,
      Abs_reciprocal_sqrt with scale 1/ls^2 on the row, then GpSimd
      partition_broadcast lands ubc = ls*u_j replicated (bf16).
    - per chunk: scr = G * ubc (one DVE pass, the only full-size 1x-rate
      pass in the kernel); exp(scr * u_i) on ScalarE with per-partition
      scale AP and accum -> sum-exp; masked sum via all-SBUF bf16 STT
      (fast DVE mode) against the bf16 mask, scaled by u_i at the end.
  * Diag exclusion: recompute the own 128x128 G block from eqc (bitwise
    identical), scale by the bitwise-identical ubc_own, extract the bf16
    diagonal exactly via identity-STT, exp with the same u_i scale ->
    subtract from sum-exp.
  * Output per core: [128, n_rt*2] f32 = (sumexp_nodiag, masked_sum).
  * Host: loss = sum(n_i*log(sumexp_i) - masked_i) / B in float64.
"""

import numpy as np
import ml_dtypes

import concourse.bass as bass
import concourse.tile as tile
from concourse import bacc, mybir
from concourse.alu_op_type import AluOpType
from concourse.bass_utils import run_bass_kernel_spmd

P = 128
CW = 1024  # column group width (2 PSUM banks of f32)
N_CORES = 8
TEMP_INV = 10.0  # 1 / temperature

F32 = mybir.dt.float32
BF16 = mybir.dt.bfloat16
FP8 = mybir.dt.float8e4

AF = mybir.ActivationFunctionType
AX = mybir.AxisListType
DRMODE = mybir.MatmulPerfMode.DoubleRow

B_FULL, D_FULL = 8192, 1024


def _build(B, D, n_cores):
    bc = B // n_cores     # rows per core
    n_rt = bc // P        # row tiles per core
    n_dt = D // P         # d tiles (must be even for DoubleRow pairs)
    n_kp = n_dt // 2      # DoubleRow k-plane pairs
    n_g = B // CW         # full-width column groups
    nb = CW // P          # 128-col blocks per group
    w_own = min(bc, CW)   # own-shard group width
    nb_own = w_own // P
    assert n_dt % 2 == 0 and B % CW == 0 and bc % P == 0 and bc <= CW
    ls = float(TEMP_INV)  # logit scale folded into ubc

    nc = bacc.Bacc(
        "TRN2", target_bir_lowering=False, debug=False, num_devices=n_cores
    )
    et8 = nc.dram_tensor("et8", (D, B), FP8, kind="ExternalInput").ap()
    etc8 = nc.dram_tensor("etc8", (D, bc), FP8, kind="ExternalInput").ap()
    maskc = nc.dram_tensor("maskc", (bc, B), BF16, kind="ExternalInput").ap()
    i128 = nc.dram_tensor("i128", (P, P), BF16, kind="ExternalInput").ap()
    stats = nc.dram_tensor("stats", (P, n_rt * 2), F32, kind="ExternalOutput").ap()

    with tile.TileContext(nc) as tc:
        with (
            tc.tile_pool(name="outer", bufs=1) as outer,
            tc.tile_pool(name="p_flat", bufs=2) as p_flat,
            tc.tile_pool(name="p_scr", bufs=3) as p_scr,
            tc.tile_pool(name="p_mask", bufs=3) as p_mask,
            tc.tile_pool(name="p_waste", bufs=1) as p_waste,
            tc.tile_pool(name="ps_sim", bufs=2, space="PSUM") as ps_simp,
            tc.tile_pool(name="ps_gram", bufs=2, space="PSUM") as ps_gramp,
        ):
            ident = outer.tile([P, P], BF16, tag="ident")
            eq = outer.tile([P, n_dt, B], FP8, tag="eq")
            eqc = outer.tile([P, n_dt, bc], FP8, tag="eqc")
            ubc = outer.tile([P, B], BF16, tag="ubc")
            ubc_own = outer.tile([P, bc], BF16, tag="ubc_own")
            ulocal = outer.tile([P, n_rt], F32, tag="ulocal")
            rdiag = outer.tile([P, n_g * nb], F32, tag="rdiag")
            rdiag_own = outer.tile([P, n_rt], F32, tag="rdiag_own")
            se_cols = outer.tile([P, n_rt * n_g], F32, tag="se_cols")
            ms_cols = outer.tile([P, n_rt * n_g], F32, tag="ms_cols")
            simii = outer.tile([P, n_rt], F32, tag="simii")
            dexp = outer.tile([P, n_rt], F32, tag="dexp")
            stats_sb = outer.tile([P, n_rt * 2], F32, tag="stats")

            nc.sync.dma_start(ident[:], i128)

            def diag_r2(src, off, blocks, rd, rd_off):
                # per 128-col block: DoubleRow gram -> diag via identity STT
                for b in range(blocks):
                    g_ps = ps_gramp.tile([P, P], F32, tag="gram")
                    for kp in range(n_kp):
                        blk = src[:, 2 * kp:2 * kp + 2,
                                  off + b * P:off + (b + 1) * P]
                        nc.tensor.matmul(
                            g_ps[:], blk, blk,
                            start=(kp == 0), stop=(kp == n_kp - 1),
                            perf_mode=DRMODE,
                        )
                    gw = p_waste.tile([P, P], BF16, tag="gramw")
                    nc.vector.scalar_tensor_tensor(
                        out=gw[:], in0=g_ps[:], scalar=1.0, in1=ident[:],
                        op0=AluOpType.mult, op1=AluOpType.mult,
                        accum_out=rd[:, rd_off + b:rd_off + b + 1],
                    )

            def bcast_u(rd, rd_off, blocks, w, dst):
                # rdiag slice [P, blocks] -> flat [1, w] row via DMA gather,
                # rsqrt on the single row, then broadcast partitions on the
                # (otherwise idle) GpSimd queue: dst = ls/sqrt(r2), replicated
                flat = p_flat.tile([1, CW], F32, tag="flat")
                for b in range(blocks):
                    nc.sync.dma_start(
                        flat[0:1, b * P:(b + 1) * P],
                        rd[:, rd_off + b:rd_off + b + 1],
                    )
                uflat = p_flat.tile([1, CW], BF16, tag="uflat")
                nc.scalar.activation(
                    uflat[0:1, :w], flat[0:1, :w], AF.Abs_reciprocal_sqrt,
                    scale=float(1.0 / (ls * ls)),
                )
                nc.gpsimd.partition_broadcast(dst, uflat[0:1, :w])

            # ---------- prologue: own shard ----------
            for dt in range(n_dt):
                nc.sync.dma_start(eqc[:, dt, :], etc8[dt * P:(dt + 1) * P, :])
            diag_r2(eqc, 0, nb_own, rdiag_own, 0)
            bcast_u(rdiag_own, 0, nb_own, w_own, ubc_own[:, :w_own])
            # ulocal = 1/sqrt(r2_own) per own row tile (plain u, no ls)
            nc.scalar.activation(
                ulocal[:], rdiag_own[:, :n_rt], AF.Abs_reciprocal_sqrt, scale=1.0
            )

            # own-block diag terms: bitwise-identical recompute
            for rt in range(n_rt):
                g_ps = ps_gramp.tile([P, P], F32, tag="gram")
                for kp in range(n_kp):
                    blk = eqc[:, 2 * kp:2 * kp + 2, rt * P:(rt + 1) * P]
                    nc.tensor.matmul(
                        g_ps[:], blk, blk,
                        start=(kp == 0), stop=(kp == n_kp - 1),
                        perf_mode=DRMODE,
                    )
                scr_own = p_scr.tile([P, P], BF16, tag="scr_own")
                nc.vector.tensor_mul(
                    scr_own[:], g_ps[:], ubc_own[:, rt * P:(rt + 1) * P]
                )
                gw = p_waste.tile([P, P], BF16, tag="gramw")
                nc.vector.scalar_tensor_tensor(
                    out=gw[:], in0=scr_own[:], scalar=1.0, in1=ident[:],
                    op0=AluOpType.mult, op1=AluOpType.mult,
                    accum_out=simii[:, rt:rt + 1],
                )
                nc.scalar.activation(
                    dexp[:, rt:rt + 1], simii[:, rt:rt + 1], AF.Exp,
                    scale=ulocal[:, rt:rt + 1],
                )

            # ---------- main: stream groups, fused r2 + gram + stats ----------
            for g in range(n_g):
                g0 = g * CW
                for dt in range(n_dt):
                    nc.sync.dma_start(
                        eq[:, dt, g0:g0 + CW],
                        et8[dt * P:(dt + 1) * P, g0:g0 + CW],
                    )
                diag_r2(eq, g0, nb, rdiag, g * nb)
                bcast_u(rdiag, g * nb, nb, CW, ubc[:, g0:g0 + CW])

                for rt in range(n_rt):
                    mask_t = p_mask.tile([P, CW], BF16, tag="mask")
                    nc.sync.dma_start(
                        mask_t[:], maskc[rt * P:(rt + 1) * P, g0:g0 + CW]
                    )
                    ps = ps_simp.tile([P, CW], F32, tag="sim")
                    for kp in range(n_kp):
                        lhs = eqc[:, 2 * kp:2 * kp + 2, rt * P:(rt + 1) * P]
                        for half in range(CW // 512):
                            nc.tensor.matmul(
                                ps[:, half * 512:(half + 1) * 512],
                                lhs,
                                eq[:, 2 * kp:2 * kp + 2,
                                   g0 + half * 512:g0 + (half + 1) * 512],
                                start=(kp == 0), stop=(kp == n_kp - 1),
                                perf_mode=DRMODE,
                            )
                    scr = p_scr.tile([P, CW], BF16, tag="scr")
                    nc.vector.tensor_mul(scr[:], ps[:], ubc[:, g0:g0 + CW])
                    ew = p_waste.tile([P, CW], BF16, tag="expw")
                    nc.scalar.activation(
                        ew[:], scr[:], AF.Exp,
                        scale=ulocal[:, rt:rt + 1],
                        accum_out=se_cols[:, rt * n_g + g:rt * n_g + g + 1],
                    )
                    mw = p_waste.tile([P, CW], BF16, tag="maskw")
                    nc.vector.scalar_tensor_tensor(
                        out=mw[:], in0=scr[:], scalar=1.0, in1=mask_t[:],
                        op0=AluOpType.mult, op1=AluOpType.mult,
                        accum_out=ms_cols[:, rt * n_g + g:rt * n_g + g + 1],
                    )

            # ---------- epilogue: per-row stats ----------
            for rt in range(n_rt):
                se_row = p_flat.tile([P, 1], F32, tag="se_row")
                nc.vector.reduce_sum(
                    se_row[:], se_cols[:, rt * n_g:(rt + 1) * n_g], axis=AX.X
                )
                nc.vector.tensor_sub(
                    stats_sb[:, rt * 2:rt * 2 + 1], se_row[:], dexp[:, rt:rt + 1]
                )
                ms_row = p_flat.tile([P, 1], F32, tag="ms_row")
                nc.vector.reduce_sum(
                    ms_row[:], ms_cols[:, rt * n_g:(rt + 1) * n_g], axis=AX.X
                )
                nc.vector.tensor_mul(
                    stats_sb[:, rt * 2 + 1:rt * 2 + 2], ms_row[:],
                    ulocal[:, rt:rt + 1],
                )
            nc.sync.dma_start(stats, stats_sb[:])

    nc.compile()
    return nc


_CACHE = {}


def _get_nc(B, D, n_cores):
    key = (B, D, n_cores)
    if key not in _CACHE:
        _CACHE[key] = _build(B, D, n_cores)
    return _CACHE[key]


def _run(embeddings, positives_mask, n_cores=N_CORES, trace=False):
    B, D = embeddings.shape
    bc = B // n_cores
    n_rt = bc // P
    nc = _get_nc(B, D, n_cores)

    et8_np = np.ascontiguousarray(
        embeddings.T.astype(np.float32)
    ).astype(ml_dtypes.float8_e4m3)
    mask_b = np.asarray(positives_mask)
    mask_bf = mask_b.astype(ml_dtypes.bfloat16)
    n_host = mask_b.sum(axis=1).astype(np.float64)
    i128 = np.eye(P, dtype=ml_dtypes.bfloat16)

    in_maps = []
    for c in range(n_cores):
        in_maps.append(
            {
                "et8": et8_np,
                "etc8": np.ascontiguousarray(et8_np[:, c * bc:(c + 1) * bc]),
                "maskc": np.ascontiguousarray(mask_bf[c * bc:(c + 1) * bc, :]),
                "i128": i128,
            }
        )

    res = run_bass_kernel_spmd(
        nc, in_maps, core_ids=list(range(n_cores)), trace=trace
    )

    total = np.float64(0.0)
    for c in range(n_cores):
        st = res.results[c]["stats"].reshape(P, n_rt, 2).astype(np.float64)
        sumexp = st[:, :, 0].T.reshape(-1)  # [bc] row-major within the core
        msum = st[:, :, 1].T.reshape(-1)
        ncnt = n_host[c * bc:(c + 1) * bc]
        total += np.sum(ncnt * np.log(sumexp) - msum)
    loss = total / B
    return np.float32(loss), res


def kernel(embeddings, positives_mask):
    loss, _ = _run(
        np.asarray(embeddings, dtype=np.float32),
        np.asarray(positives_mask),
    )
    return loss


if __name__ == "__main__":
    # small-scale smoke test against a numpy reference
    rng = np.random.default_rng(0)
    B, D = 4096, 256
    emb = rng.standard_normal((B, D), dtype=np.float32)
    mask = rng.random((B, B)) < 0.01
    np.fill_diagonal(mask, False)

    x = emb / np.maximum(np.linalg.norm(emb, axis=1, keepdims=True), 1e-12)
    sim = (x @ x.T) * TEMP_INV
    np.fill_diagonal(sim, -np.inf)
    m = sim.max(axis=1, keepdims=True)
    logz = m + np.log(np.sum(np.exp(sim - m), axis=1, keepdims=True))
    logsm = sim - logz
    expected = -np.sum(np.where(mask, logsm, 0.0)) / B

    loss, _ = _run(emb, mask, n_cores=8)
    rel = abs(loss - expected) / abs(expected)
    print(f"expected={expected:.6f} got={loss:.6f} rel_err={rel:.3e}")
